# revision 22
# baseline (speedup 1.0000x reference)
"""Masked phase-locking value (PLV) kernel for Trainium2, 8 NeuronCores.

Math: out[b] = |sum_ij M_ij * exp(i*(a_bi - b_bj))| / max(sum(M), 1)
    real_b = sum_ij M_ij (cos a_bi cos b_bj + sin a_bi sin b_bj)
    imag_b = sum_ij M_ij (sin a_bi cos b_bj - cos a_bi sin b_bj)

Device decomposition (per core, Na sharded 8 ways -> 1024 rows each):
    acc[m, j] = sum_i W[i, m] * mask[i, j]     (TensorE; W = [ca^T | sa^T], m = 2B = 128)
    racc[m]   = sum_j acc[m, j] * CS[m, j]     (DVE mult, ACT accumulate; CS = [cb; sb])
    qacc[m]   = sum_j acc[m, j] * SW[m, j]     (SW = [sb; cb], partition-swap of CS)
real_b = sum_cores racc[b] + racc[64+b]; imag_b = sum_cores qacc[64+b] - qacc[b].
All bilinear in mask rows, so Na-shard partials just add; host does the tiny
fold + |z| / sum(M).

dtypes: mask is 0/1 -> exact in fp8e4 (1 byte, halves HBM traffic, full PE rate);
weights/CS fp16 (PE full rate; end-to-end rel err ~1e-5); PSUM/epilogue fp32.
DMA order puts each mask group ahead of its trig slices so the PE starts early;
a PE warm-up burst during the DMA lead-in defeats the HAM cold-clock penalty.
"""

import numpy as np

import concourse.bass as bass
import concourse.tile as tile
from concourse import bacc, mybir
from concourse.bass_utils import run_bass_kernel_spmd

B = 64
NA = 8192
NB = 8192
NCORES = 8
NASH = NA // NCORES          # mask rows per core
KCH = NASH // 128            # contraction chunks of 128 rows
NCH = 512                    # output columns per PSUM bank
NG = 8                       # column groups (epilogue granularity)
GW = NB // NG                # group width (1024 cols = 2 PSUM banks)
NPG = GW // NCH              # psum-chunks per group

F8 = mybir.dt.float8e4
F16 = mybir.dt.float16
F32 = mybir.dt.float32


def build_program() -> bass.Bass:
    nc = bacc.Bacc("TRN2")
    mask_d = nc.dram_tensor("mask", [NG, 128, KCH, GW], F8, kind="ExternalInput")
    w_d = nc.dram_tensor("w", [128, KCH, 2 * B], F16, kind="ExternalInput")
    cs_d = nc.dram_tensor("cs", [128, NB], F16, kind="ExternalInput")
    out_d = nc.dram_tensor("out", [128, 2 * NG], F32, kind="ExternalOutput")

    copy_f = mybir.ActivationFunctionType.Copy

    with tile.TileContext(nc) as tc:
        with (
            tc.tile_pool(name="consts", bufs=1) as consts,
            tc.tile_pool(name="masks", bufs=NG) as masks,
            tc.tile_pool(name="scratch", bufs=3) as scratch,
            tc.tile_pool(name="junk", bufs=2) as junkp,
            tc.tile_pool(name="psum", bufs=3, space="PSUM") as psum_pool,
            tc.tile_pool(name="wups", bufs=1, space="PSUM") as wu_pool,
        ):
            w_sb = consts.tile([128, KCH, 2 * B], F16)
            nc.sync.dma_start(out=w_sb[:], in_=w_d[:])
            cs_sb = consts.tile([128, NB], F16)
            sw_sb = consts.tile([128, NB], F16)
            racc = consts.tile([128, 2 * NG], F32)

            # PE warm-up while the first mask group is still in flight: gets the
            # HAM clock gate to 2.4 GHz before real matmuls start. Result unused.
            wu_ps = wu_pool.tile([128, 2 * B], F32)
            for r in range(28):
                nc.tensor.matmul(
                    out=wu_ps[:],
                    lhsT=w_sb[:, 0, :],
                    rhs=w_sb[:, 1, :],
                    start=(r == 0),
                    stop=(r == 27),
                )

            for g in range(NG):
                gsl = slice(g * GW, (g + 1) * GW)
                mt = masks.tile([128, KCH, GW], F8)
                nc.sync.dma_start(out=mt[:], in_=mask_d[g])
                nc.sync.dma_start(out=cs_sb[:, gsl], in_=cs_d[:, gsl])
                # SW = partition-swapped CS slice, built on-chip via the scalar
                # HWDGE ring (SBUF->SBUF, no HBM traffic)
                nc.scalar.dma_start(out=sw_sb[:B, gsl], in_=cs_sb[B:, gsl])
                nc.scalar.dma_start(out=sw_sb[B:, gsl], in_=cs_sb[:B, gsl])

                ps = psum_pool.tile([128, GW], F32)
                for j in range(NPG):
                    jsl = slice(j * NCH, (j + 1) * NCH)
                    for k in range(KCH):
                        nc.tensor.matmul(
                            out=ps[:, jsl],
                            lhsT=w_sb[:, k, :],
                            rhs=mt[:, k, jsl],
                            start=(k == 0),
                            stop=(k == KCH - 1),
                        )
                pr = scratch.tile([128, GW], F32)
                nc.vector.tensor_mul(out=pr[:], in0=ps[:], in1=cs_sb[:, gsl])
                jr = junkp.tile([128, GW], F32)
                nc.scalar.activation(
                    out=jr[:], in_=pr[:], func=copy_f,
                    accum_out=racc[:, g : g + 1],
                )
                pi = scratch.tile([128, GW], F32)
                nc.vector.tensor_mul(out=pi[:], in0=ps[:], in1=sw_sb[:, gsl])
                ji = junkp.tile([128, GW], F32)
                nc.scalar.activation(
                    out=ji[:], in_=pi[:], func=copy_f,
                    accum_out=racc[:, NG + g : NG + g + 1],
                )

            nc.sync.dma_start(out=out_d[:], in_=racc[:])
    nc.finalize()
    return nc


def prep_inputs(phases_a, phases_b, coupling_mask):
    pa = np.asarray(phases_a, dtype=np.float32)
    pb = np.asarray(phases_b, dtype=np.float32)
    ca, sa = np.cos(pa), np.sin(pa)
    cb, sb = np.cos(pb), np.sin(pb)
    cs = np.concatenate([cb, sb], axis=0).astype(np.float16)

    f8np = mybir.dt.np(F8)
    one_byte = np.array([1.0], f8np).view(np.uint8)[0]
    mask_u8 = (np.asarray(coupling_mask) != 0).astype(np.uint8) * one_byte

    in_maps = []
    for c in range(NCORES):
        rows = slice(c * NASH, (c + 1) * NASH)
        W = np.empty((NASH, 2 * B), np.float16)
        W[:, :B] = ca[:, rows].T
        W[:, B:] = sa[:, rows].T
        # [i = k*128 + p, m] -> [p, k, m]
        w_host = np.ascontiguousarray(W.reshape(KCH, 128, 2 * B).transpose(1, 0, 2))
        # [i = k*128 + p, j = g*GW + c] -> [g, p, k, c]
        m = mask_u8[rows].reshape(KCH, 128, NG, GW).transpose(2, 1, 0, 3)
        m_host = np.ascontiguousarray(m).view(f8np)
        in_maps.append({"mask": m_host, "w": w_host, "cs": cs})
    return in_maps


def combine(outs, coupling_mask):
    o = np.stack(outs).astype(np.float64)  # [NCORES, 128, 2*NG]
    r = o[:, :, :NG].sum(axis=2)
    q = o[:, :, NG:].sum(axis=2)
    real = (r[:, :B] + r[:, B:]).sum(axis=0)
    imag = (q[:, B:] - q[:, :B]).sum(axis=0)
    n_pairs = max(float(np.asarray(coupling_mask).sum()), 1.0)
    return (np.sqrt(real * real + imag * imag) / n_pairs).astype(np.float32)


_prog_cache: list = []


def kernel(phases_a, phases_b, coupling_mask):
    in_maps = prep_inputs(phases_a, phases_b, coupling_mask)
    if not _prog_cache:
        _prog_cache.append(build_program())
    res = run_bass_kernel_spmd(_prog_cache[0], in_maps, core_ids=list(range(NCORES)))
    return combine([r["out"] for r in res.results], coupling_mask)


# revision 23
# speedup vs baseline: 1.0568x; 1.0568x over previous
"""Masked phase-locking value (PLV) kernel for Trainium2, 8 NeuronCores.

Math: out[b] = |sum_ij M_ij * exp(i*(a_bi - b_bj))| / max(sum(M), 1)
    real_b = sum_ij M_ij (cos a_bi cos b_bj + sin a_bi sin b_bj)
    imag_b = sum_ij M_ij (sin a_bi cos b_bj - cos a_bi sin b_bj)

Device decomposition (per core, Na sharded 8 ways -> 1024 rows each):
    acc[m, j] = sum_i W[i, m] * mask[i, j]     (TensorE; W = [ca^T | sa^T], m = 2B = 128)
    racc[m]   = sum_j acc[m, j] * CS[m, j]     (DVE mult, ACT accumulate; CS = [cb; sb])
    qacc[m]   = sum_j acc[m, j] * SW[m, j]     (SW = [sb; cb], partition-swap of CS)
real_b = sum_cores racc[b] + racc[64+b]; imag_b = sum_cores qacc[64+b] - qacc[b].
All bilinear in mask rows, so Na-shard partials just add; host does the tiny
fold + |z| / sum(M).

dtypes: mask is 0/1 -> exact in fp8e4 (1 byte, halves HBM traffic, full PE rate);
weights/CS fp16 (PE full rate; end-to-end rel err ~1e-5); PSUM/epilogue fp32.
DMA order puts each mask group ahead of its trig slices (trig rides the scalar
HWDGE ring) so the PE starts early; a PE warm-up burst during the DMA lead-in
defeats the HAM cold-clock penalty.
"""

import numpy as np

import concourse.bass as bass
import concourse.tile as tile
from concourse import bacc, mybir
from concourse.bass_utils import run_bass_kernel_spmd

B = 64
NA = 8192
NB = 8192
NCORES = 8
NASH = NA // NCORES          # mask rows per core
KCH = NASH // 128            # contraction chunks of 128 rows
NCH = 512                    # output columns per PSUM bank
NG = 8                       # column groups (epilogue granularity)
GW = NB // NG                # group width (1024 cols = 2 PSUM banks)
NPG = GW // NCH              # psum-chunks per group

F8 = mybir.dt.float8e4
F16 = mybir.dt.float16
F32 = mybir.dt.float32


def build_program() -> bass.Bass:
    nc = bacc.Bacc("TRN2")
    mask_d = nc.dram_tensor("mask", [NG, 128, KCH, GW], F8, kind="ExternalInput")
    w_d = nc.dram_tensor("w", [128, KCH, 2 * B], F16, kind="ExternalInput")
    cs_d = nc.dram_tensor("cs", [128, NB], F16, kind="ExternalInput")
    sw_d = nc.dram_tensor("sw", [128, NB], F16, kind="ExternalInput")
    out_d = nc.dram_tensor("out", [128, 2 * NG], F32, kind="ExternalOutput")

    copy_f = mybir.ActivationFunctionType.Copy

    with tile.TileContext(nc) as tc:
        with (
            tc.tile_pool(name="consts", bufs=1) as consts,
            tc.tile_pool(name="masks", bufs=NG) as masks,
            tc.tile_pool(name="scratch", bufs=3) as scratch,
            tc.tile_pool(name="junk", bufs=2) as junkp,
            tc.tile_pool(name="psum", bufs=3, space="PSUM") as psum_pool,
            tc.tile_pool(name="wups", bufs=1, space="PSUM") as wu_pool,
        ):
            w_sb = consts.tile([128, KCH, 2 * B], F16)
            nc.sync.dma_start(out=w_sb[:], in_=w_d[:])
            cs_sb = consts.tile([128, NB], F16)
            sw_sb = consts.tile([128, NB], F16)
            racc = consts.tile([128, 2 * NG], F32)

            # PE warm-up while the first mask group is still in flight: gets the
            # HAM clock gate to 2.4 GHz before real matmuls start. Result unused.
            wu_ps = wu_pool.tile([128, 2 * B], F32)
            for r in range(28):
                nc.tensor.matmul(
                    out=wu_ps[:],
                    lhsT=w_sb[:, 0, :],
                    rhs=w_sb[:, 1, :],
                    start=(r == 0),
                    stop=(r == 27),
                )

            for g in range(NG):
                gsl = slice(g * GW, (g + 1) * GW)
                mt = masks.tile([128, KCH, GW], F8)
                nc.sync.dma_start(out=mt[:], in_=mask_d[g])
                # trig slices on the scalar HWDGE ring: don't queue behind masks
                nc.scalar.dma_start(out=cs_sb[:, gsl], in_=cs_d[:, gsl])
                nc.scalar.dma_start(out=sw_sb[:, gsl], in_=sw_d[:, gsl])

                ps = psum_pool.tile([128, GW], F32)
                for j in range(NPG):
                    jsl = slice(j * NCH, (j + 1) * NCH)
                    for k in range(KCH):
                        nc.tensor.matmul(
                            out=ps[:, jsl],
                            lhsT=w_sb[:, k, :],
                            rhs=mt[:, k, jsl],
                            start=(k == 0),
                            stop=(k == KCH - 1),
                        )
                pr = scratch.tile([128, GW], F32)
                nc.vector.tensor_mul(out=pr[:], in0=ps[:], in1=cs_sb[:, gsl])
                jr = junkp.tile([128, GW], F32)
                nc.scalar.activation(
                    out=jr[:], in_=pr[:], func=copy_f,
                    accum_out=racc[:, g : g + 1],
                )
                pi = scratch.tile([128, GW], F32)
                nc.vector.tensor_mul(out=pi[:], in0=ps[:], in1=sw_sb[:, gsl])
                ji = junkp.tile([128, GW], F32)
                nc.scalar.activation(
                    out=ji[:], in_=pi[:], func=copy_f,
                    accum_out=racc[:, NG + g : NG + g + 1],
                )

            nc.sync.dma_start(out=out_d[:], in_=racc[:])
    nc.finalize()
    return nc


def prep_inputs(phases_a, phases_b, coupling_mask):
    pa = np.asarray(phases_a, dtype=np.float32)
    pb = np.asarray(phases_b, dtype=np.float32)
    ca, sa = np.cos(pa), np.sin(pa)
    cb, sb = np.cos(pb), np.sin(pb)
    cs = np.concatenate([cb, sb], axis=0).astype(np.float16)
    sw = np.concatenate([sb, cb], axis=0).astype(np.float16)

    f8np = mybir.dt.np(F8)
    one_byte = np.array([1.0], f8np).view(np.uint8)[0]
    mask_u8 = (np.asarray(coupling_mask) != 0).astype(np.uint8) * one_byte

    in_maps = []
    for c in range(NCORES):
        rows = slice(c * NASH, (c + 1) * NASH)
        W = np.empty((NASH, 2 * B), np.float16)
        W[:, :B] = ca[:, rows].T
        W[:, B:] = sa[:, rows].T
        # [i = k*128 + p, m] -> [p, k, m]
        w_host = np.ascontiguousarray(W.reshape(KCH, 128, 2 * B).transpose(1, 0, 2))
        # [i = k*128 + p, j = g*GW + c] -> [g, p, k, c]
        m = mask_u8[rows].reshape(KCH, 128, NG, GW).transpose(2, 1, 0, 3)
        m_host = np.ascontiguousarray(m).view(f8np)
        in_maps.append({"mask": m_host, "w": w_host, "cs": cs, "sw": sw})
    return in_maps


def combine(outs, coupling_mask):
    o = np.stack(outs).astype(np.float64)  # [NCORES, 128, 2*NG]
    r = o[:, :, :NG].sum(axis=2)
    q = o[:, :, NG:].sum(axis=2)
    real = (r[:, :B] + r[:, B:]).sum(axis=0)
    imag = (q[:, B:] - q[:, :B]).sum(axis=0)
    n_pairs = max(float(np.asarray(coupling_mask).sum()), 1.0)
    return (np.sqrt(real * real + imag * imag) / n_pairs).astype(np.float32)


_prog_cache: list = []


def kernel(phases_a, phases_b, coupling_mask):
    in_maps = prep_inputs(phases_a, phases_b, coupling_mask)
    if not _prog_cache:
        _prog_cache.append(build_program())
    res = run_bass_kernel_spmd(_prog_cache[0], in_maps, core_ids=list(range(NCORES)))
    return combine([r["out"] for r in res.results], coupling_mask)
